# revision 18
# baseline (speedup 1.0000x reference)
"""Trainium2 Bass kernel for nn_BinaryBlock (binary conv1d block).

Computation (numerically, after collapsing the STE identities):
    x_bin = where(x >= alpha, 1, -1)
    w_eff = weight * mean(|weight|, axis=(1,2), keepdims)
    y     = conv1d(x_bin, w_eff, stride 1, pad 1) + bias
    out   = RPReLU(y)  (= where(y > gamma, y - gamma + zeta, beta*(y-gamma) + zeta))

Strategy: data-parallel over batch B=32 across 8 cores (4 batches/core).
On-device, the +-1 input is recast as a {0,1} mask m = (x >= alpha):
    conv(2m-1, w) = conv(m, 2w) - S_all[co]  (+ S_k0[co] at l=0, S_k2[co] at l=L-1)
so the sign op is ONE tensor_scalar (is_ge) per x tile, and the correction
folds into the per-channel bias except for two boundary columns.

Matmuls run in fp16 at the PE roofline (1 col/cycle): the conv is 6
accumulating matmuls (2 ci-tiles x 3 taps) per [128,512] PSUM bank,
weight-major over groups of 4 l-tiles so LDWEIGHTS amortizes and PSUM
double-buffers across the 8 banks. The mask is exactly {0,1} so every
product is exact; error sources are fp16 weight rounding (~2^-11
relative) and the fp16 output store. Weights are pre-scaled by
PSCALE=128 to dodge fp16 denormals; the epilogue un-scales via the
activation's free `scale` operand.

Schedule: DMA issue costs ~0.65us per dma_start on a queue engine and
the DMA path crawls (~30GB/s) for its first few microseconds, so the
batch-0 x loads are issued first and chunked (first chunks small) on the
GpSimd queue; weights+constants are single packed DMAs on the Scalar
queue; outputs store fp16, two l-tiles per DMA, on the Sync queue. A few
discarded matmuls on a zero tile (no weight dependency) warm the PE HAM
clock during the fill so the real stream starts at full rate. Epilogues
alternate Scalar/Vector engines.
"""

import numpy as np
import ml_dtypes

# Problem shape (hardcoded per contract)
B, C, L = 32, 256, 4096
K = 3
N_CORES = 8
B_PER_CORE = B // N_CORES          # 4
P = 128                            # partitions
CI_T = C // P                      # 2 input-channel tiles
CO_T = C // P                      # 2 output-channel tiles
NT = 512                           # matmul free dim / PSUM bank (fp32)
LT = L // NT                       # 8 l-tiles
LP = L + 2                         # padded mask length
PSCALE = 128.0                     # fp16 weight pre-scale (power of 2)
GRP = 4                            # l-tiles per psum group
# batch-0 x chunk boundaries, aligned so the first (b0,co0) psum groups
# (1,1,2,4 l-tiles) consume chunks in arrival order: a group ending at
# l-tile T needs x cols through T*512
XSPLITS = (513, 1025, 2049, 3073, 4096)
WARMUP = 8                         # discarded HAM-warmup matmuls

_CACHE = {}


def _build(trivial, x_bf16_ok):
    """Build + compile the SPMD Bass program. Returns the Bacc module."""
    import concourse.bacc as bacc
    import concourse.mybir as mybir
    from concourse import tile

    f32 = mybir.dt.float32
    f16 = mybir.dt.float16
    bf16 = mybir.dt.bfloat16
    x_dt = bf16 if x_bf16_ok else f32
    Alu = mybir.AluOpType
    Act = mybir.ActivationFunctionType
    TK = CI_T * K

    nc = bacc.Bacc("TRN2", target_bir_lowering=False, debug=False,
                   num_devices=N_CORES)

    xb_d = nc.dram_tensor("xb", [B_PER_CORE * CI_T, P, L], x_dt,
                          kind="ExternalInput")
    wt_d = nc.dram_tensor("wt", [P, TK * C], f16, kind="ExternalInput")
    # cvav columns: per co_t 8 cols (0=c1, 1=sk0, 2=sk2, 3=beta-1, 4=zeta),
    # then 2 cols of alpha (per ci_t)
    cvav_d = nc.dram_tensor("cvav", [P, 2 * 8 + CI_T], f32,
                            kind="ExternalInput")
    y_d = nc.dram_tensor("y", [B_PER_CORE, CO_T, P, L], f16,
                         kind="ExternalOutput")

    with tile.TileContext(nc) as tc:
        with (
            tc.tile_pool(name="wpool", bufs=1) as wpool,
            tc.tile_pool(name="cpool", bufs=1) as cpool,
            tc.tile_pool(name="xpool", bufs=4) as xpool,
            tc.tile_pool(name="mpool", bufs=6) as mpool,
            tc.tile_pool(name="opool", bufs=8) as opool,
            tc.tile_pool(name="upool", bufs=4) as upool,
            tc.tile_pool(name="psum", bufs=8, space="PSUM") as psum,
        ):
            # ---- batch-0 x loads first, chunked; the critical chunk-1
            # pair rides the two earliest-waking queues (Scalar + Sync),
            # the rest go on GpSimd ----
            xt0 = [xpool.tile([P, L], x_dt, tag="x", name=f"x0_{ci}")
                   for ci in range(CI_T)]
            ct = cpool.tile([P, 2 * 8 + CI_T], f32, tag="cv", name="cv")
            nc.sync.dma_start(out=ct[:], in_=cvav_d[:])
            bounds = [0, *XSPLITS]
            nc.scalar.dma_start(out=xt0[0][:, 0:bounds[1]],
                                in_=xb_d[0, :, 0:bounds[1]])
            nc.sync.dma_start(out=xt0[1][:, 0:bounds[1]],
                              in_=xb_d[1, :, 0:bounds[1]])
            for c in range(1, len(XSPLITS)):
                for ci in range(CI_T):
                    lo, hi = bounds[c], bounds[c + 1]
                    nc.gpsimd.dma_start(out=xt0[ci][:, lo:hi],
                                        in_=xb_d[ci, :, lo:hi])
            # ---- weights as a single packed DMA (Scalar queue) ----
            wtile = wpool.tile([P, TK, C], f16, tag="w", name="w")
            nc.scalar.dma_start(out=wtile[:], in_=wt_d[:])
            cv_sb = [ct[:, 8 * co:8 * co + 8] for co in range(CO_T)]
            av_sb = [ct[:, 16 + ci:17 + ci] for ci in range(CI_T)]

            # ---- batch-0 masks, chunked (Vector) ----
            mt0 = [mpool.tile([P, LP], f16, tag="m", name=f"m0_{ci}")
                   for ci in range(CI_T)]
            for ci in range(CI_T):
                nc.vector.memset(mt0[ci][:, 0:1], 0.0)
                nc.vector.memset(mt0[ci][:, LP - 1:LP], 0.0)
            # zero tile for PE warmup (also on Vector, before the masks)
            if WARMUP:
                zt = mpool.tile([P, NT], f16, tag="z", name="z")
                nc.vector.memset(zt[:], 0.0)
            for c in range(len(XSPLITS)):
                for ci in range(CI_T):
                    lo, hi = bounds[c], bounds[c + 1]
                    nc.vector.tensor_scalar(
                        mt0[ci][:, 1 + lo:1 + hi], xt0[ci][:, lo:hi],
                        av_sb[ci], None, Alu.is_ge)

            # ---- PE warmup: discarded matmuls on the zero tile ----
            if WARMUP:
                wu = psum.tile([P, NT], f32, tag="ps", name="wu")
                for _ in range(WARMUP):
                    nc.tensor.matmul(wu[:], zt[:, 0:P], zt[:],
                                     start=True, stop=True)

            mt = mt0
            for b in range(B_PER_CORE):
                if b > 0:
                    mt = []
                    for ci in range(CI_T):
                        xt = xpool.tile([P, L], x_dt, tag="x", name="x")
                        nc.gpsimd.dma_start(out=xt[:],
                                            in_=xb_d[b * CI_T + ci])
                        m = mpool.tile([P, LP], f16, tag="m", name="m")
                        nc.vector.memset(m[:, 0:1], 0.0)
                        nc.vector.memset(m[:, LP - 1:LP], 0.0)
                        nc.vector.tensor_scalar(
                            m[:, 1:L + 1], xt[:], av_sb[ci], None, Alu.is_ge)
                        mt.append(m)

                for co in range(CO_T):
                    cv = cv_sb[co]
                    # small groups while batch-0 mask chunks stream in;
                    # 4-l-tile groups at steady state; tapered at the very
                    # end so the final drain is short
                    if b == 0 and co == 0:
                        grps = [1, 1, 2, 2, 2]
                    elif b == B_PER_CORE - 1 and co == CO_T - 1:
                        grps = [4, 2, 1, 1]
                    else:
                        grps = [4, 4]
                    lt0 = 0
                    for grp in grps:
                        g0 = lt0
                        lt0 += grp
                        pts = [psum.tile([P, NT], f32, tag="ps", name="ps")
                               for _ in range(grp)]
                        c = 0
                        for ci in range(CI_T):
                            for k in range(K):
                                lhsT = wtile[:, ci * K + k, co * P:(co + 1) * P]
                                for j in range(grp):
                                    s = (g0 + j) * NT + k
                                    nc.tensor.matmul(
                                        pts[j][:], lhsT, mt[ci][:, s:s + NT],
                                        start=(c == 0), stop=(c == TK - 1))
                                c += 1
                        # epilogue: alternate Scalar/Vector; 2-tile stores
                        last_grp = (b == B_PER_CORE - 1 and co == CO_T - 1
                                    and lt0 == LT)
                        stg = 1 if last_grp else min(2, grp)
                        for half in range(grp // stg):
                            ot = opool.tile([P, stg * NT], f16, tag="o",
                                            name="o")
                            for jj in range(stg):
                                j = half * stg + jj
                                l_t = g0 + j
                                dst = ot[:, jj * NT:(jj + 1) * NT]
                                if trivial:
                                    if j % 2 == 0:
                                        nc.scalar.activation(
                                            dst, pts[j][:], Act.Identity,
                                            bias=cv[:, 0:1],
                                            scale=1.0 / PSCALE)
                                    else:
                                        nc.vector.tensor_scalar(
                                            dst, pts[j][:], 1.0 / PSCALE,
                                            cv[:, 0:1], Alu.mult, Alu.add)
                                    if l_t == 0:
                                        nc.vector.tensor_scalar(
                                            ot[:, 0:1], ot[:, 0:1],
                                            cv[:, 1:2], None, Alu.add)
                                    if l_t == LT - 1:
                                        e = stg * NT
                                        nc.vector.tensor_scalar(
                                            ot[:, e - 1:e], ot[:, e - 1:e],
                                            cv[:, 2:3], None, Alu.add)
                                else:
                                    # u = psum/PSCALE + c1 (+ boundary);
                                    # out = u + zeta + (beta-1)*min(u, 0)
                                    ut = upool.tile([P, NT], f32, tag="u",
                                                    name="u")
                                    nc.scalar.activation(
                                        ut[:], pts[j][:], Act.Identity,
                                        bias=cv[:, 0:1], scale=1.0 / PSCALE)
                                    if l_t == 0:
                                        nc.vector.tensor_scalar(
                                            ut[:, 0:1], ut[:, 0:1],
                                            cv[:, 1:2], None, Alu.add)
                                    if l_t == LT - 1:
                                        nc.vector.tensor_scalar(
                                            ut[:, NT - 1:NT],
                                            ut[:, NT - 1:NT],
                                            cv[:, 2:3], None, Alu.add)
                                    nt_ = upool.tile([P, NT], f32, tag="n",
                                                     name="n")
                                    nc.vector.tensor_scalar(
                                        nt_[:], ut[:], 0.0, cv[:, 3:4],
                                        Alu.min, Alu.mult)
                                    nc.vector.tensor_scalar(
                                        ut[:], ut[:], cv[:, 4:5], None,
                                        Alu.add)
                                    nc.vector.tensor_tensor(
                                        dst, ut[:], nt_[:], Alu.add)
                            lo = (g0 + half * stg) * NT
                            if b == B_PER_CORE - 1 and co == CO_T - 1:
                                # spread the final stores across queues so
                                # the tail drain parallelizes
                                oeng = (nc.sync, nc.gpsimd)[(g0 + half) % 2]
                            else:
                                oeng = nc.sync
                            oeng.dma_start(
                                out=y_d[b, co, :, lo:lo + stg * NT],
                                in_=ot[:])

    nc.compile()
    return nc


def _host_prep(inputs):
    x = np.asarray(inputs["x"], dtype=np.float32)
    alpha = np.asarray(inputs["alpha"], dtype=np.float32).reshape(C)
    weight = np.asarray(inputs["weight"], dtype=np.float32)
    bias = np.asarray(inputs["bias"], dtype=np.float32).reshape(C)
    beta = np.asarray(inputs["beta"], dtype=np.float32).reshape(C)
    gamma = np.asarray(inputs["gamma"], dtype=np.float32).reshape(C)
    zeta = np.asarray(inputs["zeta"], dtype=np.float32).reshape(C)

    # Host-side weight prep (f32, matching the reference's f32 arithmetic)
    scale = np.mean(np.abs(weight), axis=(1, 2), dtype=np.float32)
    w_eff = weight * scale[:, None, None]              # [co, ci, k] f32
    w2 = (w_eff * (2.0 * PSCALE)).astype(np.float32)
    wT = np.ascontiguousarray(np.transpose(w2, (2, 1, 0)))  # [k, ci, co]

    # pack to [P, (ci_t*K + k, co)] with partition = ci within tile
    wt = np.ascontiguousarray(
        wT.astype(np.float16)
        .reshape(K, CI_T, P, C)
        .transpose(2, 1, 0, 3)                          # [P, ci_t, k, co]
        .reshape(P, CI_T * K * C))

    S_all = w_eff.sum(axis=(1, 2), dtype=np.float32)   # [co]
    S_k0 = w_eff[:, :, 0].sum(axis=1, dtype=np.float32)
    S_k2 = w_eff[:, :, 2].sum(axis=1, dtype=np.float32)

    trivial = bool(np.all(beta == 1.0))
    c1 = bias - gamma - S_all
    if trivial:
        c1 = c1 + zeta
    cv = np.zeros((CO_T, P, 8), dtype=np.float32)
    cv[:, :, 0] = c1.reshape(CO_T, P)
    cv[:, :, 1] = S_k0.reshape(CO_T, P)
    cv[:, :, 2] = S_k2.reshape(CO_T, P)
    cv[:, :, 3] = (beta - 1.0).reshape(CO_T, P)
    cv[:, :, 4] = zeta.reshape(CO_T, P)
    cvav = np.zeros((P, 2 * 8 + CI_T), dtype=np.float32)
    cvav[:, 0:8] = cv[0]
    cvav[:, 8:16] = cv[1]
    cvav[:, 16:16 + CI_T] = alpha.reshape(CI_T, P).T

    x_bf16_ok = bool(np.all(alpha == 0.0))
    x_dt = ml_dtypes.bfloat16 if x_bf16_ok else np.float32
    xs = x.reshape(N_CORES, B_PER_CORE * CI_T, P, L).astype(x_dt)

    in_maps = [{"xb": xs[i], "wt": wt, "cvav": cvav}
               for i in range(N_CORES)]
    return in_maps, (trivial, x_bf16_ok)


def kernel(**inputs):
    from concourse.bass_utils import run_bass_kernel_spmd

    in_maps, key = _host_prep(inputs)
    if key not in _CACHE:
        _CACHE[key] = _build(*key)
    nc = _CACHE[key]

    res = run_bass_kernel_spmd(nc, in_maps, list(range(N_CORES)))
    out = np.concatenate(
        [r["y"].reshape(B_PER_CORE, C, L) for r in res.results], axis=0)
    return out.astype(np.float32)


# revision 19
# speedup vs baseline: 1.0049x; 1.0049x over previous
"""Trainium2 Bass kernel for nn_BinaryBlock (binary conv1d block).

Computation (numerically, after collapsing the STE identities):
    x_bin = where(x >= alpha, 1, -1)
    w_eff = weight * mean(|weight|, axis=(1,2), keepdims)
    y     = conv1d(x_bin, w_eff, stride 1, pad 1) + bias
    out   = RPReLU(y)  (= where(y > gamma, y - gamma + zeta, beta*(y-gamma) + zeta))

Strategy: data-parallel over batch B=32 across 8 cores (4 batches/core).
On-device, the +-1 input is recast as a {0,1} mask m = (x >= alpha):
    conv(2m-1, w) = conv(m, 2w) - S_all[co]  (+ S_k0[co] at l=0, S_k2[co] at l=L-1)
so the sign op is ONE tensor_scalar (is_ge) per x tile, and the correction
folds into the per-channel bias except for two boundary columns.

Matmuls run in fp16 at the PE roofline (1 col/cycle): the conv is 6
accumulating matmuls (2 ci-tiles x 3 taps) per [128,512] PSUM bank,
weight-major over groups of 4 l-tiles so LDWEIGHTS amortizes and PSUM
double-buffers across the 8 banks. The mask is exactly {0,1} so every
product is exact; error sources are fp16 weight rounding (~2^-11
relative) and the fp16 output store. Weights are pre-scaled by
PSCALE=128 to dodge fp16 denormals; the epilogue un-scales via the
activation's free `scale` operand.

Schedule: DMA issue costs ~0.65us per dma_start on a queue engine and
the DMA path crawls (~30GB/s) for its first few microseconds, so the
batch-0 x loads are issued first and chunked (first chunks small) on the
GpSimd queue; weights+constants are single packed DMAs on the Scalar
queue; outputs store fp16, two l-tiles per DMA, on the Sync queue. A few
discarded matmuls on a zero tile (no weight dependency) warm the PE HAM
clock during the fill so the real stream starts at full rate. Epilogues
alternate Scalar/Vector engines.
"""

import numpy as np
import ml_dtypes

# Problem shape (hardcoded per contract)
B, C, L = 32, 256, 4096
K = 3
N_CORES = 8
B_PER_CORE = B // N_CORES          # 4
P = 128                            # partitions
CI_T = C // P                      # 2 input-channel tiles
CO_T = C // P                      # 2 output-channel tiles
NT = 512                           # matmul free dim / PSUM bank (fp32)
LT = L // NT                       # 8 l-tiles
LP = L + 2                         # padded mask length
PSCALE = 128.0                     # fp16 weight pre-scale (power of 2)
GRP = 4                            # l-tiles per psum group
# batch-0 x chunk boundaries, aligned so the first (b0,co0) psum groups
# (1,1,2,4 l-tiles) consume chunks in arrival order: a group ending at
# l-tile T needs x cols through T*512
XSPLITS = (513, 1025, 2049, 3073, 4096)
WARMUP = 8                         # discarded HAM-warmup matmuls

_CACHE = {}


def _build(trivial, x_bf16_ok):
    """Build + compile the SPMD Bass program. Returns the Bacc module."""
    import concourse.bacc as bacc
    import concourse.mybir as mybir
    from concourse import tile

    f32 = mybir.dt.float32
    f16 = mybir.dt.float16
    bf16 = mybir.dt.bfloat16
    x_dt = bf16 if x_bf16_ok else f32
    Alu = mybir.AluOpType
    Act = mybir.ActivationFunctionType
    TK = CI_T * K

    nc = bacc.Bacc("TRN2", target_bir_lowering=False, debug=False,
                   num_devices=N_CORES)

    xb_d = nc.dram_tensor("xb", [B_PER_CORE * CI_T, P, L], x_dt,
                          kind="ExternalInput")
    wt_d = nc.dram_tensor("wt", [P, TK * C], f16, kind="ExternalInput")
    # cvav columns: per co_t 8 cols (0=c1, 1=sk0, 2=sk2, 3=beta-1, 4=zeta),
    # then 2 cols of alpha (per ci_t)
    cvav_d = nc.dram_tensor("cvav", [P, 2 * 8 + CI_T], f32,
                            kind="ExternalInput")
    y_d = nc.dram_tensor("y", [B_PER_CORE, CO_T, P, L], f16,
                         kind="ExternalOutput")

    with tile.TileContext(nc) as tc:
        with (
            tc.tile_pool(name="wpool", bufs=1) as wpool,
            tc.tile_pool(name="cpool", bufs=1) as cpool,
            tc.tile_pool(name="xpool", bufs=4) as xpool,
            tc.tile_pool(name="mpool", bufs=6) as mpool,
            tc.tile_pool(name="opool", bufs=8) as opool,
            tc.tile_pool(name="upool", bufs=4) as upool,
            tc.tile_pool(name="psum", bufs=8, space="PSUM") as psum,
        ):
            # ---- batch-0 x loads first, chunked; the critical chunk-1
            # pair rides the two earliest-waking queues (Scalar + Sync),
            # the rest go on GpSimd ----
            xt0 = [xpool.tile([P, L], x_dt, tag="x", name=f"x0_{ci}")
                   for ci in range(CI_T)]
            ct = cpool.tile([P, 2 * 8 + CI_T], f32, tag="cv", name="cv")
            nc.sync.dma_start(out=ct[:], in_=cvav_d[:])
            # weights first on the Scalar queue — the real matmul stream is
            # gated on them landing by ~12us
            wtile = wpool.tile([P, TK, C], f16, tag="w", name="w")
            nc.scalar.dma_start(out=wtile[:], in_=wt_d[:])
            bounds = [0, *XSPLITS]
            nc.sync.dma_start(out=xt0[1][:, 0:bounds[1]],
                              in_=xb_d[1, :, 0:bounds[1]])
            nc.gpsimd.dma_start(out=xt0[0][:, 0:bounds[1]],
                                in_=xb_d[0, :, 0:bounds[1]])
            for c in range(1, len(XSPLITS)):
                for ci in range(CI_T):
                    lo, hi = bounds[c], bounds[c + 1]
                    nc.gpsimd.dma_start(out=xt0[ci][:, lo:hi],
                                        in_=xb_d[ci, :, lo:hi])
            cv_sb = [ct[:, 8 * co:8 * co + 8] for co in range(CO_T)]
            av_sb = [ct[:, 16 + ci:17 + ci] for ci in range(CI_T)]

            # ---- batch-0 masks, chunked (Vector) ----
            mt0 = [mpool.tile([P, LP], f16, tag="m", name=f"m0_{ci}")
                   for ci in range(CI_T)]
            for ci in range(CI_T):
                nc.vector.memset(mt0[ci][:, 0:1], 0.0)
                nc.vector.memset(mt0[ci][:, LP - 1:LP], 0.0)
            # zero tile for PE warmup (also on Vector, before the masks)
            if WARMUP:
                zt = mpool.tile([P, NT], f16, tag="z", name="z")
                nc.vector.memset(zt[:], 0.0)
            for c in range(len(XSPLITS)):
                for ci in range(CI_T):
                    lo, hi = bounds[c], bounds[c + 1]
                    nc.vector.tensor_scalar(
                        mt0[ci][:, 1 + lo:1 + hi], xt0[ci][:, lo:hi],
                        av_sb[ci], None, Alu.is_ge)

            # ---- PE warmup: discarded matmuls on the zero tile ----
            if WARMUP:
                wu = psum.tile([P, NT], f32, tag="ps", name="wu")
                for _ in range(WARMUP):
                    nc.tensor.matmul(wu[:], zt[:, 0:P], zt[:],
                                     start=True, stop=True)

            mt = mt0
            for b in range(B_PER_CORE):
                if b > 0:
                    mt = []
                    for ci in range(CI_T):
                        xt = xpool.tile([P, L], x_dt, tag="x", name="x")
                        nc.gpsimd.dma_start(out=xt[:],
                                            in_=xb_d[b * CI_T + ci])
                        m = mpool.tile([P, LP], f16, tag="m", name="m")
                        nc.vector.memset(m[:, 0:1], 0.0)
                        nc.vector.memset(m[:, LP - 1:LP], 0.0)
                        nc.vector.tensor_scalar(
                            m[:, 1:L + 1], xt[:], av_sb[ci], None, Alu.is_ge)
                        mt.append(m)

                for co in range(CO_T):
                    cv = cv_sb[co]
                    # small groups while batch-0 mask chunks stream in;
                    # 4-l-tile groups at steady state; tapered at the very
                    # end so the final drain is short
                    if b == 0 and co == 0:
                        grps = [1, 1, 2, 2, 2]
                    elif b == B_PER_CORE - 1 and co == CO_T - 1:
                        grps = [4, 2, 1, 1]
                    else:
                        grps = [4, 4]
                    lt0 = 0
                    for grp in grps:
                        g0 = lt0
                        lt0 += grp
                        pts = [psum.tile([P, NT], f32, tag="ps", name="ps")
                               for _ in range(grp)]
                        c = 0
                        for ci in range(CI_T):
                            for k in range(K):
                                lhsT = wtile[:, ci * K + k, co * P:(co + 1) * P]
                                for j in range(grp):
                                    s = (g0 + j) * NT + k
                                    nc.tensor.matmul(
                                        pts[j][:], lhsT, mt[ci][:, s:s + NT],
                                        start=(c == 0), stop=(c == TK - 1))
                                c += 1
                        # epilogue: alternate Scalar/Vector; 2-tile stores
                        last_grp = (b == B_PER_CORE - 1 and co == CO_T - 1
                                    and lt0 == LT)
                        stg = 1 if last_grp else min(2, grp)
                        for half in range(grp // stg):
                            ot = opool.tile([P, stg * NT], f16, tag="o",
                                            name="o")
                            for jj in range(stg):
                                j = half * stg + jj
                                l_t = g0 + j
                                dst = ot[:, jj * NT:(jj + 1) * NT]
                                if trivial:
                                    if j % 2 == 0:
                                        nc.scalar.activation(
                                            dst, pts[j][:], Act.Identity,
                                            bias=cv[:, 0:1],
                                            scale=1.0 / PSCALE)
                                    else:
                                        nc.vector.tensor_scalar(
                                            dst, pts[j][:], 1.0 / PSCALE,
                                            cv[:, 0:1], Alu.mult, Alu.add)
                                    if l_t == 0:
                                        nc.vector.tensor_scalar(
                                            ot[:, 0:1], ot[:, 0:1],
                                            cv[:, 1:2], None, Alu.add)
                                    if l_t == LT - 1:
                                        e = stg * NT
                                        nc.vector.tensor_scalar(
                                            ot[:, e - 1:e], ot[:, e - 1:e],
                                            cv[:, 2:3], None, Alu.add)
                                else:
                                    # u = psum/PSCALE + c1 (+ boundary);
                                    # out = u + zeta + (beta-1)*min(u, 0)
                                    ut = upool.tile([P, NT], f32, tag="u",
                                                    name="u")
                                    nc.scalar.activation(
                                        ut[:], pts[j][:], Act.Identity,
                                        bias=cv[:, 0:1], scale=1.0 / PSCALE)
                                    if l_t == 0:
                                        nc.vector.tensor_scalar(
                                            ut[:, 0:1], ut[:, 0:1],
                                            cv[:, 1:2], None, Alu.add)
                                    if l_t == LT - 1:
                                        nc.vector.tensor_scalar(
                                            ut[:, NT - 1:NT],
                                            ut[:, NT - 1:NT],
                                            cv[:, 2:3], None, Alu.add)
                                    nt_ = upool.tile([P, NT], f32, tag="n",
                                                     name="n")
                                    nc.vector.tensor_scalar(
                                        nt_[:], ut[:], 0.0, cv[:, 3:4],
                                        Alu.min, Alu.mult)
                                    nc.vector.tensor_scalar(
                                        ut[:], ut[:], cv[:, 4:5], None,
                                        Alu.add)
                                    nc.vector.tensor_tensor(
                                        dst, ut[:], nt_[:], Alu.add)
                            lo = (g0 + half * stg) * NT
                            if b == B_PER_CORE - 1 and co == CO_T - 1:
                                # spread the final stores across queues so
                                # the tail drain parallelizes
                                oeng = (nc.sync, nc.gpsimd)[(g0 + half) % 2]
                            else:
                                oeng = nc.sync
                            oeng.dma_start(
                                out=y_d[b, co, :, lo:lo + stg * NT],
                                in_=ot[:])

    nc.compile()
    return nc


def _host_prep(inputs):
    x = np.asarray(inputs["x"], dtype=np.float32)
    alpha = np.asarray(inputs["alpha"], dtype=np.float32).reshape(C)
    weight = np.asarray(inputs["weight"], dtype=np.float32)
    bias = np.asarray(inputs["bias"], dtype=np.float32).reshape(C)
    beta = np.asarray(inputs["beta"], dtype=np.float32).reshape(C)
    gamma = np.asarray(inputs["gamma"], dtype=np.float32).reshape(C)
    zeta = np.asarray(inputs["zeta"], dtype=np.float32).reshape(C)

    # Host-side weight prep (f32, matching the reference's f32 arithmetic)
    scale = np.mean(np.abs(weight), axis=(1, 2), dtype=np.float32)
    w_eff = weight * scale[:, None, None]              # [co, ci, k] f32
    w2 = (w_eff * (2.0 * PSCALE)).astype(np.float32)
    wT = np.ascontiguousarray(np.transpose(w2, (2, 1, 0)))  # [k, ci, co]

    # pack to [P, (ci_t*K + k, co)] with partition = ci within tile
    wt = np.ascontiguousarray(
        wT.astype(np.float16)
        .reshape(K, CI_T, P, C)
        .transpose(2, 1, 0, 3)                          # [P, ci_t, k, co]
        .reshape(P, CI_T * K * C))

    S_all = w_eff.sum(axis=(1, 2), dtype=np.float32)   # [co]
    S_k0 = w_eff[:, :, 0].sum(axis=1, dtype=np.float32)
    S_k2 = w_eff[:, :, 2].sum(axis=1, dtype=np.float32)

    trivial = bool(np.all(beta == 1.0))
    c1 = bias - gamma - S_all
    if trivial:
        c1 = c1 + zeta
    cv = np.zeros((CO_T, P, 8), dtype=np.float32)
    cv[:, :, 0] = c1.reshape(CO_T, P)
    cv[:, :, 1] = S_k0.reshape(CO_T, P)
    cv[:, :, 2] = S_k2.reshape(CO_T, P)
    cv[:, :, 3] = (beta - 1.0).reshape(CO_T, P)
    cv[:, :, 4] = zeta.reshape(CO_T, P)
    cvav = np.zeros((P, 2 * 8 + CI_T), dtype=np.float32)
    cvav[:, 0:8] = cv[0]
    cvav[:, 8:16] = cv[1]
    cvav[:, 16:16 + CI_T] = alpha.reshape(CI_T, P).T

    x_bf16_ok = bool(np.all(alpha == 0.0))
    x_dt = ml_dtypes.bfloat16 if x_bf16_ok else np.float32
    xs = x.reshape(N_CORES, B_PER_CORE * CI_T, P, L).astype(x_dt)

    in_maps = [{"xb": xs[i], "wt": wt, "cvav": cvav}
               for i in range(N_CORES)]
    return in_maps, (trivial, x_bf16_ok)


def kernel(**inputs):
    from concourse.bass_utils import run_bass_kernel_spmd

    in_maps, key = _host_prep(inputs)
    if key not in _CACHE:
        _CACHE[key] = _build(*key)
    nc = _CACHE[key]

    res = run_bass_kernel_spmd(nc, in_maps, list(range(N_CORES)))
    out = np.concatenate(
        [r["y"].reshape(B_PER_CORE, C, L) for r in res.results], axis=0)
    return out.astype(np.float32)


# revision 21
# speedup vs baseline: 1.0546x; 1.0494x over previous
"""Trainium2 Bass kernel for nn_BinaryBlock (binary conv1d block).

Computation (numerically, after collapsing the STE identities):
    x_bin = where(x >= alpha, 1, -1)
    w_eff = weight * mean(|weight|, axis=(1,2), keepdims)
    y     = conv1d(x_bin, w_eff, stride 1, pad 1) + bias
    out   = RPReLU(y)  (= where(y > gamma, y - gamma + zeta, beta*(y-gamma) + zeta))

Strategy: data-parallel over batch B=32 across 8 cores (4 batches/core).
On-device, the +-1 input is recast as a {0,1} mask m = (x >= alpha):
    conv(2m-1, w) = conv(m, 2w) - S_all[co]  (+ S_k0[co] at l=0, S_k2[co] at l=L-1)
so the sign op is ONE tensor_scalar (is_ge) per x tile, and the correction
folds into the per-channel bias except for two boundary columns.

Matmuls run in fp16 at the PE roofline (1 col/cycle): the conv is 6
accumulating matmuls (2 ci-tiles x 3 taps) per [128,512] PSUM bank,
weight-major over groups of 4 l-tiles so LDWEIGHTS amortizes and PSUM
double-buffers across the 8 banks. The mask is exactly {0,1} so every
product is exact; error sources are fp16 weight rounding (~2^-11
relative) and the fp16 output store. Weights are pre-scaled by
PSCALE=128 to dodge fp16 denormals; the epilogue un-scales via the
activation's free `scale` operand.

Schedule: DMA issue costs ~0.65us per dma_start on a queue engine and
the DMA path crawls (~30GB/s) for its first few microseconds, so the
batch-0 x loads are issued first and chunked (first chunks small) on the
GpSimd queue; weights+constants are single packed DMAs on the Scalar
queue; outputs store fp16, two l-tiles per DMA, on the Sync queue. A few
discarded matmuls on a zero tile (no weight dependency) warm the PE HAM
clock during the fill so the real stream starts at full rate. Epilogues
alternate Scalar/Vector engines.
"""

import numpy as np
import ml_dtypes

# Problem shape (hardcoded per contract)
B, C, L = 32, 256, 4096
K = 3
N_CORES = 8
B_PER_CORE = B // N_CORES          # 4
P = 128                            # partitions
CI_T = C // P                      # 2 input-channel tiles
CO_T = C // P                      # 2 output-channel tiles
NT = 512                           # matmul free dim / PSUM bank (fp32)
LT = L // NT                       # 8 l-tiles
LP = L + 2                         # padded mask length
PSCALE = 128.0                     # fp16 weight pre-scale (power of 2)
GRP = 4                            # l-tiles per psum group
# batch-0 x chunk boundaries, aligned so the first (b0,co0) psum groups
# (1,1,2,4 l-tiles) consume chunks in arrival order: a group ending at
# l-tile T needs x cols through T*512
XSPLITS = (513, 1025, 2049, 3073, 4096)
WARMUP = 8                         # discarded HAM-warmup matmuls

_CACHE = {}


def _build(trivial, x_bf16_ok):
    """Build + compile the SPMD Bass program. Returns the Bacc module."""
    import concourse.bacc as bacc
    import concourse.mybir as mybir
    from concourse import tile

    f32 = mybir.dt.float32
    f16 = mybir.dt.float16
    bf16 = mybir.dt.bfloat16
    x_dt = bf16 if x_bf16_ok else f32
    Alu = mybir.AluOpType
    Act = mybir.ActivationFunctionType
    TK = CI_T * K

    nc = bacc.Bacc("TRN2", target_bir_lowering=False, debug=False,
                   num_devices=N_CORES)

    xb_d = nc.dram_tensor("xb", [B_PER_CORE * CI_T, P, L], x_dt,
                          kind="ExternalInput")
    wt_d = nc.dram_tensor("wt", [P, TK * C], f16, kind="ExternalInput")
    # cvav columns: per co_t 8 cols (0=c1, 1=sk0, 2=sk2, 3=beta-1, 4=zeta),
    # then 2 cols of alpha (per ci_t)
    cvav_d = nc.dram_tensor("cvav", [P, 2 * 8 + CI_T], f32,
                            kind="ExternalInput")
    y_d = nc.dram_tensor("y", [B_PER_CORE, CO_T, P, L], f16,
                         kind="ExternalOutput")

    with tile.TileContext(nc) as tc:
        with (
            tc.tile_pool(name="wpool", bufs=1) as wpool,
            tc.tile_pool(name="cpool", bufs=1) as cpool,
            tc.tile_pool(name="xpool", bufs=4) as xpool,
            tc.tile_pool(name="mpool", bufs=6) as mpool,
            tc.tile_pool(name="opool", bufs=8) as opool,
            tc.tile_pool(name="upool", bufs=4) as upool,
            tc.tile_pool(name="psum", bufs=8, space="PSUM") as psum,
        ):
            # ---- batch-0 x loads first, chunked; the critical chunk-1
            # pair rides the two earliest-waking queues (Scalar + Sync),
            # the rest go on GpSimd ----
            # NOTE: during the DMA wake window (~first 4us of transfers)
            # bandwidth is scarce and the HW round-robins packets across
            # active queues, so keep the early critical transfers on just
            # two queues: x chunks serialized on GpSimd, weights+consts on
            # Scalar. Spreading to more queues delays the critical path.
            xt0 = [xpool.tile([P, L], x_dt, tag="x", name=f"x0_{ci}")
                   for ci in range(CI_T)]
            bounds = [0, *XSPLITS]
            for c in range(len(XSPLITS)):
                for ci in range(CI_T):
                    lo, hi = bounds[c], bounds[c + 1]
                    nc.gpsimd.dma_start(out=xt0[ci][:, lo:hi],
                                        in_=xb_d[ci, :, lo:hi])
            wtile = wpool.tile([P, TK, C], f16, tag="w", name="w")
            nc.scalar.dma_start(out=wtile[:], in_=wt_d[:])
            ct = cpool.tile([P, 2 * 8 + CI_T], f32, tag="cv", name="cv")
            nc.scalar.dma_start(out=ct[:], in_=cvav_d[:])
            cv_sb = [ct[:, 8 * co:8 * co + 8] for co in range(CO_T)]
            av_sb = [ct[:, 16 + ci:17 + ci] for ci in range(CI_T)]

            # ---- batch-0 masks, chunked (Vector) ----
            mt0 = [mpool.tile([P, LP], f16, tag="m", name=f"m0_{ci}")
                   for ci in range(CI_T)]
            for ci in range(CI_T):
                nc.vector.memset(mt0[ci][:, 0:1], 0.0)
                nc.vector.memset(mt0[ci][:, LP - 1:LP], 0.0)
            # zero tile for PE warmup (also on Vector, before the masks)
            if WARMUP:
                zt = mpool.tile([P, NT], f16, tag="z", name="z")
                nc.vector.memset(zt[:], 0.0)
            for c in range(len(XSPLITS)):
                for ci in range(CI_T):
                    lo, hi = bounds[c], bounds[c + 1]
                    nc.vector.tensor_scalar(
                        mt0[ci][:, 1 + lo:1 + hi], xt0[ci][:, lo:hi],
                        av_sb[ci], None, Alu.is_ge)

            # ---- PE warmup: discarded matmuls on the zero tile ----
            if WARMUP:
                wu = psum.tile([P, NT], f32, tag="ps", name="wu")
                for _ in range(WARMUP):
                    nc.tensor.matmul(wu[:], zt[:, 0:P], zt[:],
                                     start=True, stop=True)

            mt = mt0
            for b in range(B_PER_CORE):
                if b > 0:
                    mt = []
                    for ci in range(CI_T):
                        xt = xpool.tile([P, L], x_dt, tag="x", name="x")
                        nc.gpsimd.dma_start(out=xt[:],
                                            in_=xb_d[b * CI_T + ci])
                        m = mpool.tile([P, LP], f16, tag="m", name="m")
                        nc.vector.memset(m[:, 0:1], 0.0)
                        nc.vector.memset(m[:, LP - 1:LP], 0.0)
                        nc.vector.tensor_scalar(
                            m[:, 1:L + 1], xt[:], av_sb[ci], None, Alu.is_ge)
                        mt.append(m)

                for co in range(CO_T):
                    cv = cv_sb[co]
                    # small groups while batch-0 mask chunks stream in;
                    # 4-l-tile groups at steady state; tapered at the very
                    # end so the final drain is short
                    if b == 0 and co == 0:
                        grps = [1, 1, 2, 2, 2]
                    elif b == B_PER_CORE - 1 and co == CO_T - 1:
                        grps = [4, 2, 1, 1]
                    else:
                        grps = [4, 4]
                    lt0 = 0
                    for grp in grps:
                        g0 = lt0
                        lt0 += grp
                        pts = [psum.tile([P, NT], f32, tag="ps", name="ps")
                               for _ in range(grp)]
                        c = 0
                        for ci in range(CI_T):
                            for k in range(K):
                                lhsT = wtile[:, ci * K + k, co * P:(co + 1) * P]
                                for j in range(grp):
                                    s = (g0 + j) * NT + k
                                    nc.tensor.matmul(
                                        pts[j][:], lhsT, mt[ci][:, s:s + NT],
                                        start=(c == 0), stop=(c == TK - 1))
                                c += 1
                        # epilogue: alternate Scalar/Vector; 2-tile stores
                        last_grp = (b == B_PER_CORE - 1 and co == CO_T - 1
                                    and lt0 == LT)
                        stg = 1 if last_grp else min(2, grp)
                        for half in range(grp // stg):
                            ot = opool.tile([P, stg * NT], f16, tag="o",
                                            name="o")
                            for jj in range(stg):
                                j = half * stg + jj
                                l_t = g0 + j
                                dst = ot[:, jj * NT:(jj + 1) * NT]
                                if trivial:
                                    if j % 2 == 0:
                                        nc.scalar.activation(
                                            dst, pts[j][:], Act.Identity,
                                            bias=cv[:, 0:1],
                                            scale=1.0 / PSCALE)
                                    else:
                                        nc.vector.tensor_scalar(
                                            dst, pts[j][:], 1.0 / PSCALE,
                                            cv[:, 0:1], Alu.mult, Alu.add)
                                    if l_t == 0:
                                        nc.vector.tensor_scalar(
                                            ot[:, 0:1], ot[:, 0:1],
                                            cv[:, 1:2], None, Alu.add)
                                    if l_t == LT - 1:
                                        e = stg * NT
                                        nc.vector.tensor_scalar(
                                            ot[:, e - 1:e], ot[:, e - 1:e],
                                            cv[:, 2:3], None, Alu.add)
                                else:
                                    # u = psum/PSCALE + c1 (+ boundary);
                                    # out = u + zeta + (beta-1)*min(u, 0)
                                    ut = upool.tile([P, NT], f32, tag="u",
                                                    name="u")
                                    nc.scalar.activation(
                                        ut[:], pts[j][:], Act.Identity,
                                        bias=cv[:, 0:1], scale=1.0 / PSCALE)
                                    if l_t == 0:
                                        nc.vector.tensor_scalar(
                                            ut[:, 0:1], ut[:, 0:1],
                                            cv[:, 1:2], None, Alu.add)
                                    if l_t == LT - 1:
                                        nc.vector.tensor_scalar(
                                            ut[:, NT - 1:NT],
                                            ut[:, NT - 1:NT],
                                            cv[:, 2:3], None, Alu.add)
                                    nt_ = upool.tile([P, NT], f32, tag="n",
                                                     name="n")
                                    nc.vector.tensor_scalar(
                                        nt_[:], ut[:], 0.0, cv[:, 3:4],
                                        Alu.min, Alu.mult)
                                    nc.vector.tensor_scalar(
                                        ut[:], ut[:], cv[:, 4:5], None,
                                        Alu.add)
                                    nc.vector.tensor_tensor(
                                        dst, ut[:], nt_[:], Alu.add)
                            lo = (g0 + half * stg) * NT
                            nc.sync.dma_start(
                                out=y_d[b, co, :, lo:lo + stg * NT],
                                in_=ot[:])

    nc.compile()
    return nc


def _host_prep(inputs):
    x = np.asarray(inputs["x"], dtype=np.float32)
    alpha = np.asarray(inputs["alpha"], dtype=np.float32).reshape(C)
    weight = np.asarray(inputs["weight"], dtype=np.float32)
    bias = np.asarray(inputs["bias"], dtype=np.float32).reshape(C)
    beta = np.asarray(inputs["beta"], dtype=np.float32).reshape(C)
    gamma = np.asarray(inputs["gamma"], dtype=np.float32).reshape(C)
    zeta = np.asarray(inputs["zeta"], dtype=np.float32).reshape(C)

    # Host-side weight prep (f32, matching the reference's f32 arithmetic)
    scale = np.mean(np.abs(weight), axis=(1, 2), dtype=np.float32)
    w_eff = weight * scale[:, None, None]              # [co, ci, k] f32
    w2 = (w_eff * (2.0 * PSCALE)).astype(np.float32)
    wT = np.ascontiguousarray(np.transpose(w2, (2, 1, 0)))  # [k, ci, co]

    # pack to [P, (ci_t*K + k, co)] with partition = ci within tile
    wt = np.ascontiguousarray(
        wT.astype(np.float16)
        .reshape(K, CI_T, P, C)
        .transpose(2, 1, 0, 3)                          # [P, ci_t, k, co]
        .reshape(P, CI_T * K * C))

    S_all = w_eff.sum(axis=(1, 2), dtype=np.float32)   # [co]
    S_k0 = w_eff[:, :, 0].sum(axis=1, dtype=np.float32)
    S_k2 = w_eff[:, :, 2].sum(axis=1, dtype=np.float32)

    trivial = bool(np.all(beta == 1.0))
    c1 = bias - gamma - S_all
    if trivial:
        c1 = c1 + zeta
    cv = np.zeros((CO_T, P, 8), dtype=np.float32)
    cv[:, :, 0] = c1.reshape(CO_T, P)
    cv[:, :, 1] = S_k0.reshape(CO_T, P)
    cv[:, :, 2] = S_k2.reshape(CO_T, P)
    cv[:, :, 3] = (beta - 1.0).reshape(CO_T, P)
    cv[:, :, 4] = zeta.reshape(CO_T, P)
    cvav = np.zeros((P, 2 * 8 + CI_T), dtype=np.float32)
    cvav[:, 0:8] = cv[0]
    cvav[:, 8:16] = cv[1]
    cvav[:, 16:16 + CI_T] = alpha.reshape(CI_T, P).T

    x_bf16_ok = bool(np.all(alpha == 0.0))
    x_dt = ml_dtypes.bfloat16 if x_bf16_ok else np.float32
    xs = x.reshape(N_CORES, B_PER_CORE * CI_T, P, L).astype(x_dt)

    in_maps = [{"xb": xs[i], "wt": wt, "cvav": cvav}
               for i in range(N_CORES)]
    return in_maps, (trivial, x_bf16_ok)


def kernel(**inputs):
    from concourse.bass_utils import run_bass_kernel_spmd

    in_maps, key = _host_prep(inputs)
    if key not in _CACHE:
        _CACHE[key] = _build(*key)
    nc = _CACHE[key]

    res = run_bass_kernel_spmd(nc, in_maps, list(range(N_CORES)))
    out = np.concatenate(
        [r["y"].reshape(B_PER_CORE, C, L) for r in res.results], axis=0)
    return out.astype(np.float32)
